# revision 6
# baseline (speedup 1.0000x reference)
"""Causal single-head attention (B=4, S=2048, D=DK=1024) on 8 trn2 NeuronCores.

Sharding: data-parallel over batch x interleaved q-blocks. Core c handles
batch b=c//2, parity p=c%2, owning the 8 q-blocks {2j+p : j in 0..7} (128 rows
each). One uniform SPMD program runs on all 8 cores; per-core differences are
carried entirely by the input data (host-side column permutation + mask tiles).

Math per core (weight-folded to skip full-context K/V projections):
    Q^T = W_Q^T X_q^T                 [dk, 1024]
    G^T = (Q W_K^T)^T = W_K Q^T ... computed as lhsT=W_K^T tiles  [d, 1024]
    S   = G X_ctx^T   (causal window, compact 2-region layout)
    A   = softmax(S/32 with -1e9 mask pre-scale)
    P   = A X_ctx     (bf16)
    out = P W_V       (then scatter rows back on host)

Matmuls run as float32r (full PE rate at N>=256, ~tf32 precision); the A@X
stage runs bf16. Host pre-transposes X^T / W_K^T so no fp32 DMA transposes are
needed on-chip.
"""

import numpy as np
import ml_dtypes

B, S, D = 4, 2048, 1024
P = 128               # partitions
NJ = 8                # q-tiles per core
NCORES = 8
MASK_FILL = -1.0e9

_cache = {}


def _build_program():
    from contextlib import ExitStack
    import concourse.bass as bass
    import concourse.bacc as bacc
    import concourse.tile as tile
    import concourse.mybir as mybir
    from concourse import masks

    f32 = mybir.dt.float32
    f32r = mybir.dt.float32r
    bf16 = mybir.dt.bfloat16
    Exp = mybir.ActivationFunctionType.Exp
    AX = mybir.AxisListType.X
    ts = bass.ts

    nc = bacc.Bacc("TRN2", target_bir_lowering=False, debug=False,
                   enable_asserts=False)

    xct_d = nc.dram_tensor("xct", [D, S], f32r, kind="ExternalInput").ap()
    xc_d = nc.dram_tensor("xc", [S, D], bf16, kind="ExternalInput").ap()
    wq_d = nc.dram_tensor("wq", [D, D], f32r, kind="ExternalInput").ap()
    wkt_d = nc.dram_tensor("wkt", [D, D], f32r, kind="ExternalInput").ap()
    wv_d = nc.dram_tensor("wv", [D, D], f32r, kind="ExternalInput").ap()
    madd_d = nc.dram_tensor("madd", [NJ * P, 2 * P], f32,
                            kind="ExternalInput").ap()
    out_d = nc.dram_tensor("out", [NJ * P, D], f32, kind="ExternalOutput").ap()

    xct_r = xct_d.rearrange("(c p) k -> c p k", p=P)    # [8, 128, 2048]
    xc_r = xc_d.rearrange("(c p) d -> c p d", p=P)      # [16, 128, 1024]
    wq_r = wq_d.rearrange("(c p) n -> c p n", p=P)
    wkt_r = wkt_d.rearrange("(c p) n -> c p n", p=P)
    wv_r = wv_d.rearrange("(c p) n -> c p n", p=P)

    with tile.TileContext(nc) as tc, ExitStack() as es:
        # ---- persistent pools -------------------------------------------
        perm = es.enter_context(tc.tile_pool(name="perm", bufs=1))
        xct_sb = perm.tile([P, 8, S], f32r)        # X_ctx^T  64KB/part
        xc_sb = perm.tile([P, 16, D], bf16)       # X_ctx (perm rows) 32KB/part
        gt_sb = perm.tile([P, 8, 1024], f32r)      # G^T 32KB/part
        ident_b = perm.tile([P, P], bf16)
        ident_f = perm.tile([P, P], f32)

        masks.make_identity(nc, ident_b[:])
        masks.make_identity(nc, ident_f[:])

        # persistent input DMAs (chunked so consumers unblock early)
        for dc in range(8):
            nc.sync.dma_start(xct_sb[:, dc, 0:1024], xct_r[dc, :, 0:1024])
        for dc in range(8):
            nc.sync.dma_start(xct_sb[:, dc, 1024:2048], xct_r[dc, :, 1024:2048])
        for kb in range(16):
            nc.sync.dma_start(xc_sb[:, kb, :], xc_r[kb])

        # ---- phase A: Q^T = W_Q^T X_q^T ---------------------------------
        qt_pool = tc.alloc_tile_pool(name="qt", bufs=1)
        qt_sb = qt_pool.tile([P, 8, 1024], f32r)

        with tc.tile_pool(name="wq", bufs=1) as wqp, \
             tc.tile_pool(name="pps", bufs=4, space="PSUM") as pps:
            wq_sb = wqp.tile([P, 8, 1024], f32r)
            for dc in range(8):
                nc.sync.dma_start(wq_sb[:, dc, :], wq_r[dc])
            for dkt in range(8):
                for qh in (0, 512):
                    ps = pps.tile([P, 512], f32, tag="ps")
                    for dc in range(8):
                        nc.tensor.matmul(
                            ps[:], wq_sb[:, dc, ts(dkt, P)],
                            xct_sb[:, dc, qh:qh + 512],
                            start=(dc == 0), stop=(dc == 7))
                    nc.vector.tensor_copy(qt_sb[:, dkt, qh:qh + 512], ps[:])

        # ---- phase B: G^T = W_K^T-tiles x Q^T ---------------------------
        with tc.tile_pool(name="wkt", bufs=1) as wktp, \
             tc.tile_pool(name="pps2", bufs=4, space="PSUM") as pps2:
            wkt_sb = wktp.tile([P, 8, 1024], f32r)
            for dc in range(8):
                nc.sync.dma_start(wkt_sb[:, dc, :], wkt_r[dc])
            for dt_ in range(8):
                for qh in (0, 512):
                    ps = pps2.tile([P, 512], f32, tag="ps")
                    for kc in range(8):
                        nc.tensor.matmul(
                            ps[:], wkt_sb[:, kc, ts(dt_, P)],
                            qt_sb[:, kc, qh:qh + 512],
                            start=(kc == 0), stop=(kc == 7))
                    nc.vector.tensor_copy(gt_sb[:, dt_, qh:qh + 512], ps[:])
        qt_pool.release()

        # ---- phase D: attention per q-tile ------------------------------
        with tc.tile_pool(name="wv", bufs=1) as wvp, \
             tc.tile_pool(name="work1", bufs=1) as work1, \
             tc.tile_pool(name="work2", bufs=2) as work2, \
             tc.tile_pool(name="stats", bufs=4) as statp, \
             tc.tile_pool(name="sps", bufs=2, space="PSUM") as spsp, \
             tc.tile_pool(name="trp", bufs=2, space="PSUM") as trp, \
             tc.tile_pool(name="ppp", bufs=2, space="PSUM") as ppp, \
             tc.tile_pool(name="ops", bufs=2, space="PSUM") as opsp:
            wv_sb = wvp.tile([P, 8, 1024], f32r)
            for dc in range(8):
                nc.sync.dma_start(wv_sb[:, dc, :], wv_r[dc])

            for j in range(NJ):
                nk = 2 * j + 2          # 128-wide k-chunks this q-tile
                W = nk * P              # compact context width
                hw = (j + 1) * P        # per-region width

                madd_t = work2.tile([P, 2 * P], f32, tag="madd")
                nc.sync.dma_start(madd_t[:], madd_d[ts(j, P), :])

                srow = work1.tile([P, 2048], f32, tag="srow")
                # two column regions: own q-blocks [0:1024), others [1024:2048)
                for base_src, base_dst in ((0, 0), (1024, hw)):
                    for off in range(0, hw, 512):
                        w = min(512, hw - off)
                        ps = spsp.tile([P, 512], f32, tag="ps")
                        for dc in range(8):
                            nc.tensor.matmul(
                                ps[:, :w], gt_sb[:, dc, ts(j, P)],
                                xct_sb[:, dc,
                                       base_src + off:base_src + off + w],
                                start=(dc == 0), stop=(dc == 7))
                        nc.vector.tensor_copy(
                            srow[:, base_dst + off:base_dst + off + w],
                            ps[:, :w])
                # additive causal mask on the two boundary chunks
                nc.vector.tensor_add(srow[:, ts(j, P)], srow[:, ts(j, P)],
                                     madd_t[:, 0:P])
                nc.vector.tensor_add(srow[:, ts(2 * j + 1, P)],
                                     srow[:, ts(2 * j + 1, P)],
                                     madd_t[:, P:2 * P])

                mx = statp.tile([P, 1], f32, tag="mx")
                nc.vector.reduce_max(mx[:], srow[:, :W], axis=AX)
                nmx = statp.tile([P, 1], f32, tag="nmx")
                nc.scalar.mul(nmx[:], mx[:], -1.0 / 32.0)
                sumexp = statp.tile([P, 1], f32, tag="se")
                attn = work1.tile([P, 2048], bf16, tag="attn")
                nc.scalar.activation(attn[:, :W], srow[:, :W], Exp,
                                     bias=nmx[:], scale=1.0 / 32.0,
                                     accum_out=sumexp[:])
                rcp = statp.tile([P, 1], f32, tag="rcp")
                nc.vector.reciprocal(rcp[:], sumexp[:])

                attnT = work1.tile([P, 2048], bf16, tag="attnT")
                for c in range(nk):
                    tp = trp.tile([P, P], bf16, tag="tr")
                    nc.tensor.transpose(tp[:], attn[:, ts(c, P)], ident_b[:])
                    nc.vector.tensor_copy(attnT[:, ts(c, P)], tp[:])

                p_sb = work2.tile([P, 1024], f32, tag="p")
                for dh in (0, 512):
                    pp = ppp.tile([P, 512], f32, tag="pp")
                    for c in range(nk):
                        pos = c if c <= j else 8 + (c - j - 1)
                        nc.tensor.matmul(
                            pp[:], attnT[:, ts(c, P)],
                            xc_sb[:, pos, dh:dh + 512],
                            start=(c == 0), stop=(c == nk - 1))
                    nc.vector.tensor_copy(p_sb[:, dh:dh + 512], pp[:])

                pt_sb = work2.tile([P, 1024], f32r, tag="pt")
                for dc in range(8):
                    tp = trp.tile([P, P], f32, tag="tr")
                    nc.tensor.transpose(tp[:], p_sb[:, ts(dc, P)], ident_f[:])
                    nc.vector.tensor_copy(pt_sb[:, ts(dc, P)], tp[:])

                out_sb = work2.tile([P, 1024], f32, tag="out")
                for dvh in (0, 512):
                    op = opsp.tile([P, 512], f32, tag="op")
                    for dc in range(8):
                        nc.tensor.matmul(
                            op[:], pt_sb[:, ts(dc, P)],
                            wv_sb[:, dc, dvh:dvh + 512],
                            start=(dc == 0), stop=(dc == 7))
                    # normalize by softmax denominator during PSUM->SBUF copy
                    nc.scalar.activation(
                        out_sb[:, dvh:dvh + 512], op[:],
                        mybir.ActivationFunctionType.Copy, scale=rcp[:])
                nc.sync.dma_start(out_d[ts(j, P), :], out_sb[:])

    nc.compile()
    return nc


def _prep_inputs(sequence_repr, W_Q, W_K, W_V, mask):
    """Build the 8 per-core input dicts (host-side slicing/permutation)."""
    wkt = np.ascontiguousarray(W_K.T)
    in_maps = []
    meta = []
    for c in range(NCORES):
        b, par = divmod(c, 2)
        qblocks = [2 * j + par for j in range(NJ)]
        oblocks = [2 * j + 1 - par for j in range(NJ)]
        posblocks = qblocks + oblocks
        rows_perm = np.concatenate(
            [np.arange(g * P, (g + 1) * P) for g in posblocks])
        qrows = rows_perm[:NJ * P]
        xb = sequence_repr[b]
        xct = np.ascontiguousarray(xb.T[:, rows_perm])
        xc = np.ascontiguousarray(xb[rows_perm]).astype(ml_dtypes.bfloat16)
        madd = np.empty((NJ * P, 2 * P), np.float32)
        for j in range(NJ):
            g = 2 * j + par
            gb = 2 * j + 1 - par
            qr = slice((2 * j + par) * P, (2 * j + par) * P + P)
            madd[j * P:(j + 1) * P, 0:P] = np.where(
                mask[b, qr, g * P:(g + 1) * P], 0.0, MASK_FILL)
            madd[j * P:(j + 1) * P, P:2 * P] = np.where(
                mask[b, qr, gb * P:(gb + 1) * P], 0.0, MASK_FILL)
        in_maps.append({
            "xct": xct, "xc": xc,
            "wq": np.ascontiguousarray(W_Q),
            "wkt": wkt,
            "wv": np.ascontiguousarray(W_V),
            "madd": madd,
        })
        meta.append((b, qrows))
    return in_maps, meta


def run(sequence_repr, W_Q, W_K, W_V, mask, trace=False):
    from concourse.bass_utils import run_bass_kernel_spmd

    if "nc" not in _cache:
        _cache["nc"] = _build_program()
    nc = _cache["nc"]
    in_maps, meta = _prep_inputs(
        np.asarray(sequence_repr, np.float32), np.asarray(W_Q, np.float32),
        np.asarray(W_K, np.float32), np.asarray(W_V, np.float32),
        np.asarray(mask))
    res = run_bass_kernel_spmd(nc, in_maps, core_ids=list(range(NCORES)),
                               trace=trace)
    out = np.empty((B, S, D), np.float32)
    for c in range(NCORES):
        b, qrows = meta[c]
        out[b, qrows] = res.results[c]["out"]
    return out, res


def kernel(**inputs):
    out, _ = run(**inputs)
    return out


# revision 10
# speedup vs baseline: 1.1337x; 1.1337x over previous
"""Causal single-head attention (B=4, S=2048, D=DK=1024) on 8 trn2 NeuronCores.

Sharding: data-parallel over batch x interleaved q-blocks. Core c handles
batch b=c//2, parity p=c%2, owning the 8 q-blocks {2j+p : j in 0..7} (128 rows
each). One uniform SPMD program runs on all 8 cores; per-core differences are
carried entirely by the input data (host-side column permutation + mask tiles).

Math per core (weight-folded to skip full-context K/V projections):
    Q^T = W_Q^T X_q^T                 [dk, 1024]
    G^T = (Q W_K^T)^T = W_K Q^T ... computed as lhsT=W_K^T tiles  [d, 1024]
    S   = G X_ctx^T   (causal window, compact 2-region layout)
    A   = softmax(S/32 with -1e9 mask pre-scale)
    P   = A X_ctx     (bf16)
    out = P W_V       (then scatter rows back on host)

Matmuls run as float32r (full PE rate at N>=256, ~tf32 precision); the A@X
stage runs bf16. Host pre-transposes X^T / W_K^T so no fp32 DMA transposes are
needed on-chip.
"""

import numpy as np
import ml_dtypes

B, S, D = 4, 2048, 1024
P = 128               # partitions
NJ = 8                # q-tiles per core
NCORES = 8
MASK_FILL = -1.0e9

_cache = {}


def _build_program():
    from contextlib import ExitStack
    import concourse.bass as bass
    import concourse.bacc as bacc
    import concourse.tile as tile
    import concourse.mybir as mybir
    from concourse import masks

    f32 = mybir.dt.float32
    f32r = mybir.dt.float32r
    bf16 = mybir.dt.bfloat16
    Exp = mybir.ActivationFunctionType.Exp
    AX = mybir.AxisListType.X
    ts = bass.ts

    nc = bacc.Bacc("TRN2", target_bir_lowering=False, debug=False,
                   enable_asserts=False)

    xct_d = nc.dram_tensor("xct", [D, S], f32r, kind="ExternalInput").ap()
    xc_d = nc.dram_tensor("xc", [S, D], bf16, kind="ExternalInput").ap()
    wq_d = nc.dram_tensor("wq", [D, D], f32r, kind="ExternalInput").ap()
    wkt_d = nc.dram_tensor("wkt", [D, D], f32r, kind="ExternalInput").ap()
    wv_d = nc.dram_tensor("wv", [D, D], f32r, kind="ExternalInput").ap()
    madd_d = nc.dram_tensor("madd", [NJ * P, 2 * P], f32,
                            kind="ExternalInput").ap()
    out_d = nc.dram_tensor("out", [NJ * P, D], f32, kind="ExternalOutput").ap()

    xct_r = xct_d.rearrange("(c p) k -> c p k", p=P)    # [8, 128, 2048]
    xc_r = xc_d.rearrange("(c p) d -> c p d", p=P)      # [16, 128, 1024]
    wq_r = wq_d.rearrange("(c p) n -> c p n", p=P)
    wkt_r = wkt_d.rearrange("(c p) n -> c p n", p=P)
    wv_r = wv_d.rearrange("(c p) n -> c p n", p=P)

    with tile.TileContext(nc) as tc, ExitStack() as es:
        # ---- persistent pools -------------------------------------------
        perm = es.enter_context(tc.tile_pool(name="perm", bufs=1))
        xct_sb = perm.tile([P, 8, S], f32r)        # X_ctx^T  64KB/part
        xc_sb = perm.tile([P, 16, D], bf16)       # X_ctx (perm rows) 32KB/part
        gt_sb = perm.tile([P, 8, 1024], f32r)      # G^T 32KB/part
        ident_b = perm.tile([P, P], bf16)
        ident_f = perm.tile([P, P], f32)

        masks.make_identity(nc, ident_b[:])
        masks.make_identity(nc, ident_f[:])

        # ---- phase A: Q^T = W_Q^T X_q^T ---------------------------------
        # dc-outer accumulation into 8 live PSUM banks so the first matmul
        # only needs the first wq/xct chunk; DMAs are issued in first-use
        # order (wq chunk k interleaved with the xct q-half chunk k).
        qt_pool = tc.alloc_tile_pool(name="qt", bufs=1)
        qt_sb = qt_pool.tile([P, 8, 1024], f32r)

        with tc.tile_pool(name="wq", bufs=1) as wqp, \
             tc.tile_pool(name="pps", bufs=8, space="PSUM") as pps:
            wq_sb = wqp.tile([P, 8, 1024], f32r)
            for dc in range(8):
                nc.sync.dma_start(wq_sb[:, dc, :], wq_r[dc])
                nc.sync.dma_start(xct_sb[:, dc, 0:1024], xct_r[dc, :, 0:1024])
            for half in range(2):
                dkts = range(half * 4, half * 4 + 4)
                psl = {(dkt, qh): pps.tile([P, 512], f32, tag="ps",
                                           name=f"psA{dkt}{qh}")
                       for dkt in dkts for qh in (0, 512)}
                for dc in range(8):
                    for dkt in dkts:
                        for qh in (0, 512):
                            nc.tensor.matmul(
                                psl[(dkt, qh)][:], wq_sb[:, dc, ts(dkt, P)],
                                xct_sb[:, dc, qh:qh + 512],
                                start=(dc == 0), stop=(dc == 7))
                for dkt in dkts:
                    for qh in (0, 512):
                        nc.vector.tensor_copy(qt_sb[:, dkt, qh:qh + 512],
                                              psl[(dkt, qh)][:])

        # remaining persistent inputs (needed from phase D onward)
        for dc in range(8):
            nc.sync.dma_start(xct_sb[:, dc, 1024:2048], xct_r[dc, :, 1024:2048])
        for kb in range(16):
            nc.sync.dma_start(xc_sb[:, kb, :], xc_r[kb])

        # ---- phase B: G^T = W_K^T-tiles x Q^T ---------------------------
        # wkt is streamed per-contraction-chunk (2 passes over 4 d'-tiles
        # each) so wv can be resident during B and prefetch under it.
        wv_pool = tc.alloc_tile_pool(name="wv", bufs=1, side="right")
        wv_sb = wv_pool.tile([P, 8, 1024], f32r)
        for dc in range(8):
            nc.sync.dma_start(wv_sb[:, dc, :], wv_r[dc])

        with tc.tile_pool(name="wkts", bufs=3) as wktsp, \
             tc.tile_pool(name="pps2", bufs=8, space="PSUM") as pps2:
            for half in range(2):
                dts = range(half * 4, half * 4 + 4)
                psl = {(dt_, qh): pps2.tile([P, 512], f32, tag="ps",
                                            name=f"psB{dt_}{qh}")
                       for dt_ in dts for qh in (0, 512)}
                for kc in range(8):
                    wkc = wktsp.tile([P, 1024], f32r, tag="wk", name="wkc")
                    nc.sync.dma_start(wkc[:], wkt_r[kc])
                    for dt_ in dts:
                        for qh in (0, 512):
                            nc.tensor.matmul(
                                psl[(dt_, qh)][:], wkc[:, ts(dt_, P)],
                                qt_sb[:, kc, qh:qh + 512],
                                start=(kc == 0), stop=(kc == 7))
                for dt_ in dts:
                    for qh in (0, 512):
                        nc.vector.tensor_copy(gt_sb[:, dt_, qh:qh + 512],
                                              psl[(dt_, qh)][:])
        qt_pool.release()

        # ---- phase D: attention per q-tile ------------------------------
        with tc.tile_pool(name="work1", bufs=1) as work1, \
             tc.tile_pool(name="work2", bufs=2) as work2, \
             tc.tile_pool(name="stats", bufs=4) as statp, \
             tc.tile_pool(name="sps", bufs=2, space="PSUM") as spsp, \
             tc.tile_pool(name="trp", bufs=2, space="PSUM") as trp, \
             tc.tile_pool(name="ppp", bufs=2, space="PSUM") as ppp, \
             tc.tile_pool(name="ops", bufs=2, space="PSUM") as opsp:
            for j in range(NJ):
                nk = 2 * j + 2          # 128-wide k-chunks this q-tile
                W = nk * P              # compact context width
                hw = (j + 1) * P        # per-region width

                madd_t = work2.tile([P, 2 * P], f32, tag="madd")
                nc.sync.dma_start(madd_t[:], madd_d[ts(j, P), :])

                srow = work1.tile([P, 2048], f32, tag="srow")
                # two column regions: own q-blocks [0:1024), others [1024:2048)
                for base_src, base_dst in ((0, 0), (1024, hw)):
                    for off in range(0, hw, 512):
                        w = min(512, hw - off)
                        ps = spsp.tile([P, 512], f32, tag="ps")
                        for dc in range(8):
                            nc.tensor.matmul(
                                ps[:, :w], gt_sb[:, dc, ts(j, P)],
                                xct_sb[:, dc,
                                       base_src + off:base_src + off + w],
                                start=(dc == 0), stop=(dc == 7))
                        nc.vector.tensor_copy(
                            srow[:, base_dst + off:base_dst + off + w],
                            ps[:, :w])
                # additive causal mask on the two boundary chunks
                nc.vector.tensor_add(srow[:, ts(j, P)], srow[:, ts(j, P)],
                                     madd_t[:, 0:P])
                nc.vector.tensor_add(srow[:, ts(2 * j + 1, P)],
                                     srow[:, ts(2 * j + 1, P)],
                                     madd_t[:, P:2 * P])

                mx = statp.tile([P, 1], f32, tag="mx")
                nc.vector.reduce_max(mx[:], srow[:, :W], axis=AX)
                nmx = statp.tile([P, 1], f32, tag="nmx")
                nc.scalar.mul(nmx[:], mx[:], -1.0 / 32.0)
                sumexp = statp.tile([P, 1], f32, tag="se")
                attn = work1.tile([P, 2048], bf16, tag="attn")
                nc.scalar.activation(attn[:, :W], srow[:, :W], Exp,
                                     bias=nmx[:], scale=1.0 / 32.0,
                                     accum_out=sumexp[:])
                rcp = statp.tile([P, 1], f32, tag="rcp")
                nc.vector.reciprocal(rcp[:], sumexp[:])

                attnT = work1.tile([P, 2048], bf16, tag="attnT")
                for c in range(nk):
                    tp = trp.tile([P, P], bf16, tag="tr")
                    nc.tensor.transpose(tp[:], attn[:, ts(c, P)], ident_b[:])
                    nc.vector.tensor_copy(attnT[:, ts(c, P)], tp[:])

                p_sb = work2.tile([P, 1024], f32, tag="p")
                for dh in (0, 512):
                    pp = ppp.tile([P, 512], f32, tag="pp")
                    for c in range(nk):
                        pos = c if c <= j else 8 + (c - j - 1)
                        nc.tensor.matmul(
                            pp[:], attnT[:, ts(c, P)],
                            xc_sb[:, pos, dh:dh + 512],
                            start=(c == 0), stop=(c == nk - 1))
                    nc.vector.tensor_copy(p_sb[:, dh:dh + 512], pp[:])

                pt_sb = work2.tile([P, 1024], f32r, tag="pt")
                for dc in range(8):
                    tp = trp.tile([P, P], f32, tag="tr")
                    nc.tensor.transpose(tp[:], p_sb[:, ts(dc, P)], ident_f[:])
                    nc.vector.tensor_copy(pt_sb[:, ts(dc, P)], tp[:])

                out_sb = work2.tile([P, 1024], f32, tag="out")
                for dvh in (0, 512):
                    op = opsp.tile([P, 512], f32, tag="op")
                    for dc in range(8):
                        nc.tensor.matmul(
                            op[:], pt_sb[:, ts(dc, P)],
                            wv_sb[:, dc, dvh:dvh + 512],
                            start=(dc == 0), stop=(dc == 7))
                    # normalize by softmax denominator during PSUM->SBUF copy
                    nc.scalar.activation(
                        out_sb[:, dvh:dvh + 512], op[:],
                        mybir.ActivationFunctionType.Copy, scale=rcp[:])
                nc.sync.dma_start(out_d[ts(j, P), :], out_sb[:])
        wv_pool.release()

    nc.compile()
    return nc


def _prep_inputs(sequence_repr, W_Q, W_K, W_V, mask):
    """Build the 8 per-core input dicts (host-side slicing/permutation)."""
    wkt = np.ascontiguousarray(W_K.T)
    in_maps = []
    meta = []
    for c in range(NCORES):
        b, par = divmod(c, 2)
        qblocks = [2 * j + par for j in range(NJ)]
        oblocks = [2 * j + 1 - par for j in range(NJ)]
        posblocks = qblocks + oblocks
        rows_perm = np.concatenate(
            [np.arange(g * P, (g + 1) * P) for g in posblocks])
        qrows = rows_perm[:NJ * P]
        xb = sequence_repr[b]
        xct = np.ascontiguousarray(xb.T[:, rows_perm])
        xc = np.ascontiguousarray(xb[rows_perm]).astype(ml_dtypes.bfloat16)
        madd = np.empty((NJ * P, 2 * P), np.float32)
        for j in range(NJ):
            g = 2 * j + par
            gb = 2 * j + 1 - par
            qr = slice((2 * j + par) * P, (2 * j + par) * P + P)
            madd[j * P:(j + 1) * P, 0:P] = np.where(
                mask[b, qr, g * P:(g + 1) * P], 0.0, MASK_FILL)
            madd[j * P:(j + 1) * P, P:2 * P] = np.where(
                mask[b, qr, gb * P:(gb + 1) * P], 0.0, MASK_FILL)
        in_maps.append({
            "xct": xct, "xc": xc,
            "wq": np.ascontiguousarray(W_Q),
            "wkt": wkt,
            "wv": np.ascontiguousarray(W_V),
            "madd": madd,
        })
        meta.append((b, qrows))
    return in_maps, meta


def run(sequence_repr, W_Q, W_K, W_V, mask, trace=False):
    from concourse.bass_utils import run_bass_kernel_spmd

    if "nc" not in _cache:
        _cache["nc"] = _build_program()
    nc = _cache["nc"]
    in_maps, meta = _prep_inputs(
        np.asarray(sequence_repr, np.float32), np.asarray(W_Q, np.float32),
        np.asarray(W_K, np.float32), np.asarray(W_V, np.float32),
        np.asarray(mask))
    res = run_bass_kernel_spmd(nc, in_maps, core_ids=list(range(NCORES)),
                               trace=trace)
    out = np.empty((B, S, D), np.float32)
    for c in range(NCORES):
        b, qrows = meta[c]
        out[b, qrows] = res.results[c]["out"]
    return out, res


def kernel(**inputs):
    out, _ = run(**inputs)
    return out
